# revision 8
# baseline (speedup 1.0000x reference)
"""DifferentialAttention Trainium2 kernel (8-core SPMD), v2.

Sharding: 8 cores = 4 batches x 2 head-groups (8 heads each).
v2 changes vs baseline:
  - single 3D-AP DMA loads for x/wq/wk/wv/wc, bitcast f32 -> f32r for
    matmuls (no cast copies, ~90 fewer DMA issues per iteration)
  - QT/KT/p/vaug in bf16 (same PE rate, half SBUF; p tiles double-buffered
    so exp(j) never waits on u(j-1) reads)
  - softmax denominator broadcast via ones-column PE matmul into PSUM
    (replaces per-head DMA round-trips through DRAM)
  - GroupNorm without per-head Ln/Exp table thrash: normalize by
    z=1/(var+eps) (DVE reciprocal), fold sqrt(var+eps)*(1-lam_init) into a
    one-time row-scale of Wc before the output projection
  - per-head stats/normalize in the pipeline; head order ends on an even
    slot so no partition-shift DMA gates the output projection
  - p-tile causal zeros + constants hoisted out of the timing loop
"""

import math
import sys

for _p in ("/opt/trn_rl_repo", "/root/.axon_site/_ro/trn_rl_repo"):
    if _p not in sys.path:
        sys.path.append(_p)

from contextlib import ExitStack

import ml_dtypes
import numpy as np

BF16_NP = ml_dtypes.bfloat16

import concourse.mybir as mybir
import concourse.tile as tile
from concourse import bacc
from concourse.bass_utils import run_bass_kernel_spmd

F32 = mybir.dt.float32
F32R = mybir.dt.float32r
BF16 = mybir.dt.bfloat16
AF = mybir.ActivationFunctionType
OP = mybir.AluOpType

B, T, C = 4, 1024, 1024
NH = 16
HD = C // NH  # 64
NHL = 8  # heads per core
LAMBDA_INIT = 0.8 - 0.6 * math.exp(-0.3 * 1.0)
EPS = 1e-5
SCALE = 1.0 / math.sqrt(HD)
N_CORES = 8
NKT = T // 128  # 8 token tiles
NKC = C // 128  # 8 contraction tiles
HEAD_ORDER = [1, 3, 5, 7, 0, 2, 4, 6]  # odd slots first: no DMA on the tail


def _const(nc, val, shape):
    return nc.const_aps.tensor(val, shape)


def _dram_tensors(nc):
    x_d = nc.dram_tensor("xbT", [C, T], BF16, kind="ExternalInput").ap()
    wq_d = nc.dram_tensor("wq", [C, 1024], BF16, kind="ExternalInput").ap()
    wk_d = nc.dram_tensor("wk", [C, 1024], BF16, kind="ExternalInput").ap()
    wv_d = nc.dram_tensor("wv", [C, 512], BF16, kind="ExternalInput").ap()
    wc_d = nc.dram_tensor("wc", [512, C], F32, kind="ExternalInput").ap()
    neglam_d = nc.dram_tensor("neglam", [1, 64], F32, kind="ExternalInput").ap()
    out_d = nc.dram_tensor("outp", [T, C], F32, kind="ExternalOutput").ap()
    return (x_d, wq_d, wk_d, wv_d, wc_d, neglam_d, out_d)


def build_program(n_iters: int = 1):
    nc = bacc.Bacc("TRN2", target_bir_lowering=False, debug=False)
    dram = _dram_tensors(nc)

    with tile.TileContext(nc) as tc, ExitStack() as bctx:
        pers = _alloc_persistent(nc, tc, bctx, dram)
        if n_iters == 1:
            _emit_iteration(nc, tc, pers, dram)
        else:
            with tc.For_i(0, n_iters, 1):
                _emit_iteration(nc, tc, pers, dram)

    nc.compile()
    return nc


def _alloc_persistent(nc, tc, bctx, dram):
    """Pools + one-time init (consts, causal-zero p regions, vaug ones col)."""
    neglam_d = dram[5]
    lp = bctx.enter_context(tc.tile_pool(name="long", bufs=1))
    qk = bctx.enter_context(tc.tile_pool(name="qk", bufs=1))
    oe = bctx.enter_context(tc.tile_pool(name="oe", bufs=1))
    yout = bctx.enter_context(tc.tile_pool(name="yn", bufs=1))
    pp = bctx.enter_context(tc.tile_pool(name="pp", bufs=1))

    p = {"lp": lp, "qk": qk, "oe": oe, "yout": yout, "pp": pp}

    p["neglam_f"] = lp.tile([1, 64], F32, tag="neglam_f", name="neglam_f")
    # lambda is iteration-invariant: load once
    p["onesr"] = lp.tile([1, 64], F32R, tag="onesr", name="onesr")
    nc.vector.tensor_copy(p["onesr"][:], _const(nc, 1.0, (1, 64)))
    p["actscr"] = lp.tile([1, 1], F32, tag="actscr", name="actscr")
    p["ones128"] = lp.tile([64, 128], F32R, tag="ones128", name="ones128")
    nc.vector.tensor_copy(p["ones128"][:], _const(nc, 1.0, (64, 128)))
    nc.sync.dma_start(p["neglam_f"][:], neglam_d)
    p["neglamr"] = lp.tile([1, 64], F32R, tag="neglamr", name="neglamr")
    nc.vector.tensor_copy(p["neglamr"][:], p["neglam_f"][:])

    p["wc_t"] = oe.tile([128, 4, C], F32, tag="wc_t", name="wc_t")
    p["wcs_t"] = oe.tile([128, 4, C], F32R, tag="wcs_t", name="wcs_t")
    p["vaug"] = [
        lp.tile([128, NHL, HD + 1], BF16, tag=f"vaug{t}", name=f"vaug{t}") for t in range(NKT)
    ]
    for tt in range(NKT):
        nc.vector.tensor_copy(p["vaug"][tt][:, :, HD : HD + 1], _const(nc, 1.0, (128, NHL, 1)))
    p["QT"] = [qk.tile([128, T], BF16, tag=f"qt{m}", name=f"qt{m}") for m in range(NKC)]
    p["KT"] = [qk.tile([128, T], BF16, tag=f"kt{m}", name=f"kt{m}") for m in range(NKC)]
    p["yTn"] = [yout.tile([128, T], F32R, tag=f"ytn{k}", name=f"ytn{k}") for k in range(4)]

    # p tiles: explicit double buffer (parity = head % 2), bf16.
    # Region [0, d0) is the causal-mask zero block: written only here, read by
    # every u matmul, so the zeros persist across loop iterations.
    p["pbuf"] = []
    for par in range(2):
        p1 = [
            pp.tile([128, T - (i // 4) * 512], BF16, tag=f"p1_{par}_{i}", name=f"p1_{par}_{i}")
            for i in range(NKT)
        ]
        p2 = [
            pp.tile([128, T - (i // 4) * 512], BF16, tag=f"p2_{par}_{i}", name=f"p2_{par}_{i}")
            for i in range(NKT)
        ]
        p["pbuf"].append((p1, p2))
        for i in range(NKT):
            d0 = i * 128 - (i // 4) * 512
            if d0 > 0:
                nc.vector.memset(p1[i][:, 0:d0], 0.0)
                nc.vector.memset(p2[i][:, 0:d0], 0.0)

    # x / projection weights: persistent bf16 tiles so the next iteration's
    # input DMAs only wait on this iteration's projection reads (prefetch)
    xw = bctx.enter_context(tc.tile_pool(name="xw", bufs=1))
    p["x_t"] = xw.tile([128, NKC, T], BF16, tag="x_t", name="x_t")
    p["wqk_t"] = xw.tile([128, NKC, 1024], BF16, tag="wqk_t", name="wqk_t")
    p["wv_t"] = xw.tile([128, NKC, 512], BF16, tag="wv_t", name="wv_t")
    return p


def _emit_iteration(nc, tc, pers, dram):
    x_d, wq_d, wk_d, wv_d, wc_d, neglam_d, out_d = dram
    QT, KT, vaug, yTn = pers["QT"], pers["KT"], pers["vaug"], pers["yTn"]
    wc_t, wcs_t, neglamr = pers["wc_t"], pers["wcs_t"], pers["neglamr"]
    onesr, ones128, pbuf = pers["onesr"], pers["ones128"], pers["pbuf"]

    # preload the exp table off the critical path (ACT idles during phase A/B)
    nc.scalar.activation(pers["actscr"][:], onesr[0:1, 0:1], AF.Exp)

    # ---------------- phase A+B: projections (Q -> V -> K) ----------------
    with ExitStack() as ab:
        psb = ab.enter_context(tc.tile_pool(name="psb", bufs=3, space="PSUM"))

        x_t, wqk_t, wv_t = pers["x_t"], pers["wqk_t"], pers["wv_t"]
        x_r = x_d.rearrange("(k p) n -> p k n", p=128)
        nc.sync.dma_start(x_t[:, 0:4, :], x_r[:, 0:4, :])
        nc.sync.dma_start(x_t[:, 4:8, :], x_r[:, 4:8, :])

        wq_r = wq_d.rearrange("(k p) n -> p k n", p=128)
        nc.sync.dma_start(wqk_t[:, 0:4, :], wq_r[:, 0:4, :])
        nc.sync.dma_start(wqk_t[:, 4:8, :], wq_r[:, 4:8, :])
        nc.sync.dma_start(wv_t[:], wv_d.rearrange("(k p) n -> p k n", p=128))
        # wc after wv: its WAR (previous out-proj) resolves at the iteration
        # boundary, so it must not head-of-line block the x/wq prefetch
        nc.sync.dma_start(wc_t[:], wc_d.rearrange("(k p) n -> p k n", p=128))

        def proj_qk(dest):
            for m in range(NKC):
                pq = psb.tile([128, T], F32, tag="proj", bufs=3)
                for c0 in range(0, T, 512):
                    for k in range(NKC):
                        nc.tensor.matmul(
                            pq[:, c0 : c0 + 512],
                            wqk_t[:, k, m * 128 : (m + 1) * 128],
                            x_t[:, k, c0 : c0 + 512],
                            start=(k == 0),
                            stop=(k == NKC - 1),
                        )
                if m % 2 == 0:
                    nc.scalar.copy(dest[m][:], pq[:])
                else:
                    nc.vector.tensor_copy(dest[m][:], pq[:])

        proj_qk(QT)

        # V projection into Vaug while wk streams in
        wk_r = wk_d.rearrange("(k p) n -> p k n", p=128)
        for tt in range(NKT):
            pv = psb.tile([128, 512], F32, tag="projv", bufs=2)
            for k in range(NKC):
                nc.tensor.matmul(
                    pv[:],
                    x_t[:, k, tt * 128 : (tt + 1) * 128],
                    wv_t[:, k, :],
                    start=(k == 0),
                    stop=(k == NKC - 1),
                )
            if tt % 2 == 0:
                nc.vector.tensor_copy(
                    vaug[tt][:, :, 0:HD], pv[:].rearrange("p (h d) -> p h d", h=NHL)
                )
            else:
                nc.scalar.copy(
                    vaug[tt][:, :, 0:HD], pv[:].rearrange("p (h d) -> p h d", h=NHL)
                )
        nc.sync.dma_start(wqk_t[:, 0:4, :], wk_r[:, 0:4, :])
        nc.sync.dma_start(wqk_t[:, 4:8, :], wk_r[:, 4:8, :])

        proj_qk(KT)

    # ---------------- phase C: attention per head ----------------
    with ExitStack() as cc_:
        yt = cc_.enter_context(tc.tile_pool(name="yt", bufs=1))
        sm = cc_.enter_context(tc.tile_pool(name="sm", bufs=2))
        pss = cc_.enter_context(tc.tile_pool(name="pss", bufs=1, space="PSUM"))
        psu = cc_.enter_context(tc.tile_pool(name="psu", bufs=1, space="PSUM"))
        psr = cc_.enter_context(tc.tile_pool(name="psr", bufs=1, space="PSUM"))

        # varW[p, kk] = var+eps of head 2*kk + p//64, replicated in its slot
        varW = sm.tile([128, 4], F32, tag="varW", bufs=1)
        yT_heads = {}

        def emit_score_mms(j, i):
            c0 = (i // 4) * 512
            w = T - c0
            s1 = pss.tile([128, w], F32, tag="s1", name=f"s1_{j}_{i}")
            s2 = pss.tile([128, w], F32, tag="s2", name=f"s2_{j}_{i}")
            for cb in range(0, w, 512):
                nc.tensor.matmul(
                    s1[:, cb : cb + 512],
                    KT[j][0:64, i * 128 : (i + 1) * 128],
                    QT[j][0:64, c0 + cb : c0 + cb + 512],
                    start=True,
                    stop=True,
                    tile_position=(0, 0),
                )
                nc.tensor.matmul(
                    s2[:, cb : cb + 512],
                    KT[j][64:128, i * 128 : (i + 1) * 128],
                    QT[j][64:128, c0 + cb : c0 + cb + 512],
                    start=True,
                    stop=True,
                    tile_position=(64, 0),
                )
            return s1, s2

        def emit_scores(j, par, s_pre=None):
            p1_t, p2_t = pbuf[par]
            for i in range(NKT):
                c0 = (i // 4) * 512
                d0 = i * 128 - c0
                if i == 0 and s_pre is not None:
                    s1, s2 = s_pre
                else:
                    s1, s2 = emit_score_mms(j, i)
                nc.scalar.activation(p1_t[i][:, d0:], s1[:, d0:], AF.Exp, scale=SCALE)
                nc.scalar.activation(p2_t[i][:, d0:], s2[:, d0:], AF.Exp, scale=SCALE)
                for pt in (p1_t, p2_t):
                    nc.gpsimd.affine_select(
                        out=pt[i][:, d0 : d0 + 128],
                        in_=pt[i][:, d0 : d0 + 128],
                        compare_op=OP.is_ge,
                        fill=0.0,
                        base=0,
                        pattern=[[1, 128]],
                        channel_multiplier=-1,
                    )
            yT_heads[j] = yt.tile([64, T], F32, tag="yT", bufs=3, name=f"yTh{j}")

        def emit_u(j, par, c):
            p1, p2 = pbuf[par]
            yT_h = yT_heads[j]
            ilast = min(NKT, (c + 1) * 4) - 1
            u1 = psu.tile([HD + 1, 512], F32, tag="u1", name=f"u1_{j}_{c}")
            u2 = psu.tile([HD + 1, 512], F32, tag="u2", name=f"u2_{j}_{c}")
            for i in range(ilast + 1):
                lo = c * 512 - (i // 4) * 512
                nc.tensor.matmul(
                    u1[:], vaug[i][:, j, :], p1[i][:, lo : lo + 512],
                    start=(i == 0), stop=(i == ilast),
                )
            for i in range(ilast + 1):
                lo = c * 512 - (i // 4) * 512
                nc.tensor.matmul(
                    u2[:], vaug[i][:, j, :], p2[i][:, lo : lo + 512],
                    start=(i == 0), stop=(i == ilast),
                )
            rr1 = sm.tile([1, 512], F32R, tag="rr1", bufs=1, name=f"rr1_{j}_{c}")
            rr2 = sm.tile([1, 512], F32R, tag="rr2", bufs=1, name=f"rr2_{j}_{c}")
            with nc.allow_low_precision(reason="f32r denominators feed a broadcast matmul"):
                nc.vector.reciprocal(rr1[:], u1[64:65, :])
                nc.vector.reciprocal(rr2[:], u2[64:65, :])
            # broadcast 1/den across 64 partitions via contraction-1 matmul;
            # -lam folds into the u2 broadcast via the neglam stationary
            R1s = psr.tile([64, 512], F32, tag="R1", name=f"R1_{j}_{c}")
            R2s = psr.tile([64, 512], F32, tag="R2", name=f"R2_{j}_{c}")
            nc.tensor.matmul(R1s[:], onesr[:], rr1[:], start=True, stop=True)
            nc.tensor.matmul(R2s[:], neglamr[:], rr2[:], start=True, stop=True)
            # TensorTensor may read at most one PSUM input: stage R in SBUF
            R1b = sm.tile([64, 512], F32, tag="R1b", bufs=1, name=f"R1b_{j}_{c}")
            R2b = sm.tile([64, 512], F32, tag="R2b", bufs=1, name=f"R2b_{j}_{c}")
            nc.scalar.copy(R1b[:], R1s[:])
            nc.scalar.copy(R2b[:], R2s[:])
            t1 = sm.tile([64, 512], F32, tag="t1", bufs=1, name=f"t1_{j}_{c}")
            t2 = sm.tile([64, 512], F32, tag="t2", bufs=1, name=f"t2_{j}_{c}")
            nc.vector.tensor_tensor(t1[:], u1[0:HD, :], R1b[:], OP.mult)
            nc.vector.tensor_tensor(t2[:], u2[0:HD, :], R2b[:], OP.mult)
            # gpsimd cannot read PSUM; it gets the SBUF-only add
            nc.gpsimd.tensor_tensor(yT_h[:, c * 512 : (c + 1) * 512], t1[:], t2[:], OP.add)

        def emit_stats_norm(j):
            yT_h = yT_heads[j]
            bstats = sm.tile([64, 2, 6], F32, tag="bst", name=f"bst_{j}")
            for si in range(2):
                nc.vector.bn_stats(out=bstats[:, si, :], in_=yT_h[:, si * 512 : (si + 1) * 512])
            mv = sm.tile([64, 2], F32, tag="mv", name=f"mv_{j}")
            nc.vector.bn_aggr(out=mv[:], in_=bstats[:])
            # st = (mean, var + mean^2) per partition; sum across partitions
            st = sm.tile([64, 2], F32R, tag="st", name=f"st_{j}")
            m2p = sm.tile([64, 1], F32, tag="m2p", name=f"m2p_{j}")
            nc.vector.tensor_tensor(m2p[:], mv[:, 0:1], mv[:, 0:1], OP.mult)
            nc.vector.tensor_tensor(st[:, 1:2], mv[:, 1:2], m2p[:], OP.add)
            nc.vector.tensor_copy(st[:, 0:1], mv[:, 0:1])
            pstat = psu.tile([128, 2], F32, tag="u1", name=f"pstat_{j}")
            nc.tensor.matmul(pstat[:], ones128[:], st[:], start=True, stop=True)
            nm128 = sm.tile([128, 1], F32, tag="nm128", name=f"nm_{j}")
            nc.vector.tensor_scalar_mul(nm128[:], pstat[:, 0:1], -1.0 / 64.0)
            m2 = sm.tile([128, 1], F32, tag="m2", name=f"m2_{j}")
            nc.vector.tensor_tensor(m2[:], nm128[:], nm128[:], OP.mult)
            ve = sm.tile([128, 1], F32, tag="ve", name=f"ve_{j}")
            nc.vector.tensor_scalar(
                out=ve[:], in0=pstat[:, 1:2], scalar1=1.0 / 64.0, scalar2=EPS,
                op0=OP.mult, op1=OP.add,
            )
            nc.vector.tensor_tensor(ve[:], ve[:], m2[:], OP.subtract)
            lo = (j % 2) * 64
            nc.vector.tensor_copy(varW[lo : lo + 64, j // 2 : j // 2 + 1], ve[lo : lo + 64, :])
            z = sm.tile([64, 1], F32, tag="z", name=f"z_{j}")
            nc.vector.reciprocal(z[:], ve[0:64, :])
            # normalize to (y - mean)/(var+eps); the sqrt(var+eps)*(1-lam_init)
            # factor lands in the Wc row scale at the end
            if j % 2 == 0:
                nc.vector.tensor_scalar(
                    out=yTn[j // 2][0:64, :], in0=yT_h[:],
                    scalar1=nm128[0:64, :], scalar2=z[:], op0=OP.add, op1=OP.mult,
                )
            else:
                ymv = sm.tile([64, T], F32R, tag="ymv", bufs=1, name=f"ymv_{j}")
                nc.vector.tensor_scalar(
                    out=ymv[:], in0=yT_h[:],
                    scalar1=nm128[0:64, :], scalar2=z[:], op0=OP.add, op1=OP.mult,
                )
                nc.scalar.dma_start(yTn[j // 2][64:128, :], ymv[:])

        # software-pipelined head loop
        for idx, j in enumerate(HEAD_ORDER):
            par = idx % 2
            s_pre = None
            if idx > 0:
                s_pre = emit_score_mms(j, 0)
                emit_u(HEAD_ORDER[idx - 1], 1 - par, 1)
            emit_scores(j, par, s_pre)
            emit_u(j, par, 0)
            if idx > 0:
                emit_stats_norm(HEAD_ORDER[idx - 1])
        emit_u(HEAD_ORDER[-1], 1, 1)
        # fold sqrt(var+eps)*(1-lam_init) into Wc rows. Heads for kk=0..2 are
        # done; sqrt them (one table load) + scale on DVE while the last
        # head's u/stats run, leaving only kk=3 on the tail.
        c2 = (1.0 - LAMBDA_INIT) ** 2
        srstd = sm.tile([128, 4], F32, tag="srstd", bufs=1)
        nc.scalar.activation(srstd[:, 0:3], varW[:, 0:3], AF.Sqrt, scale=c2)
        for kk in range(3):
            nc.vector.tensor_scalar_mul(wcs_t[:, kk, :], wc_t[:, kk, :], srstd[:, kk : kk + 1])
        emit_stats_norm(HEAD_ORDER[-1])
        nc.scalar.activation(srstd[:, 3:4], varW[:, 3:4], AF.Sqrt, scale=c2)
        nc.vector.tensor_scalar_mul(wcs_t[:, 3, :], wc_t[:, 3, :], srstd[:, 3:4])

    # ---------------- phase E: output projection ----------------
    with ExitStack() as ee:
        ob = ee.enter_context(tc.tile_pool(name="ob", bufs=4))
        pso = ee.enter_context(tc.tile_pool(name="pso", bufs=3, space="PSUM"))
        for m in range(NKC):
            po = pso.tile([128, C], F32, tag="o")
            for c0 in range(0, C, 512):
                for kk in range(4):
                    nc.tensor.matmul(
                        po[:, c0 : c0 + 512],
                        yTn[kk][:, m * 128 : (m + 1) * 128],
                        wcs_t[:, kk, c0 : c0 + 512],
                        start=(kk == 0),
                        stop=(kk == 3),
                    )
            osb = ob.tile([128, C], F32, tag="osb")
            if m % 2 == 0:
                nc.vector.tensor_copy(osb[:], po[:])
            else:
                nc.scalar.copy(osb[:], po[:])
            # output stores ride the gpsimd queue (idle at phase E) so they
            # neither block the SP prefetch nor delay the last osb copies
            nc.gpsimd.dma_start(out_d[m * 128 : (m + 1) * 128, :], osb[:])


_PROGRAM_CACHE = {}


def get_program(n_iters: int = 1):
    if n_iters not in _PROGRAM_CACHE:
        _PROGRAM_CACHE[n_iters] = build_program(n_iters)
    return _PROGRAM_CACHE[n_iters]


def make_in_maps(x, Wq, Wk, Wv, Wc, lambda_q1, lambda_k1, lambda_q2, lambda_k2):
    lam = (
        math.exp(float(np.sum(lambda_q1.astype(np.float64) * lambda_k1.astype(np.float64))))
        - math.exp(float(np.sum(lambda_q2.astype(np.float64) * lambda_k2.astype(np.float64))))
        + LAMBDA_INIT
    )
    neglam = np.full((1, 64), -lam, dtype=np.float32)
    in_maps = []
    for core in range(N_CORES):
        b, g = core // 2, core % 2
        in_maps.append(
            {
                "xbT": np.ascontiguousarray(x[b].T).astype(BF16_NP),
                "wq": np.ascontiguousarray(Wq[:, g * 1024 : (g + 1) * 1024]).astype(BF16_NP),
                "wk": np.ascontiguousarray(Wk[:, g * 1024 : (g + 1) * 1024]).astype(BF16_NP),
                "wv": np.ascontiguousarray(Wv[:, g * 512 : (g + 1) * 512]).astype(BF16_NP),
                "wc": np.ascontiguousarray(Wc[g * 512 : (g + 1) * 512, :]),
                "neglam": neglam,
            }
        )
    return in_maps


def kernel(x, Wq, Wk, Wv, Wc, lambda_q1, lambda_k1, lambda_q2, lambda_k2):
    x = np.asarray(x, dtype=np.float32)
    in_maps = make_in_maps(
        x,
        np.asarray(Wq, np.float32),
        np.asarray(Wk, np.float32),
        np.asarray(Wv, np.float32),
        np.asarray(Wc, np.float32),
        np.asarray(lambda_q1, np.float32),
        np.asarray(lambda_k1, np.float32),
        np.asarray(lambda_q2, np.float32),
        np.asarray(lambda_k2, np.float32),
    )
    nc = get_program(1)
    res = run_bass_kernel_spmd(nc, in_maps, list(range(N_CORES)))
    out = np.empty((B, T, C), dtype=np.float32)
    for b in range(B):
        out[b] = res.results[2 * b]["outp"] + res.results[2 * b + 1]["outp"]
    return out


# revision 9
# speedup vs baseline: 1.1033x; 1.1033x over previous
"""DifferentialAttention Trainium2 kernel (8-core SPMD), v2.

Sharding: 8 cores = 4 batches x 2 head-groups (8 heads each).
v2 changes vs baseline:
  - single 3D-AP DMA loads for x/wq/wk/wv/wc, bitcast f32 -> f32r for
    matmuls (no cast copies, ~90 fewer DMA issues per iteration)
  - QT/KT/p/vaug in bf16 (same PE rate, half SBUF; p tiles double-buffered
    so exp(j) never waits on u(j-1) reads)
  - softmax denominator broadcast via ones-column PE matmul into PSUM
    (replaces per-head DMA round-trips through DRAM)
  - GroupNorm without per-head Ln/Exp table thrash: normalize by
    z=1/(var+eps) (DVE reciprocal), fold sqrt(var+eps)*(1-lam_init) into a
    one-time row-scale of Wc before the output projection
  - per-head stats/normalize in the pipeline; head order ends on an even
    slot so no partition-shift DMA gates the output projection
  - p-tile causal zeros + constants hoisted out of the timing loop
"""

import math
import sys

for _p in ("/opt/trn_rl_repo", "/root/.axon_site/_ro/trn_rl_repo"):
    if _p not in sys.path:
        sys.path.append(_p)

from contextlib import ExitStack

import ml_dtypes
import numpy as np

BF16_NP = ml_dtypes.bfloat16

import concourse.mybir as mybir
import concourse.tile as tile
from concourse import bacc
from concourse.bass_utils import run_bass_kernel_spmd

F32 = mybir.dt.float32
F32R = mybir.dt.float32r
BF16 = mybir.dt.bfloat16
AF = mybir.ActivationFunctionType
OP = mybir.AluOpType

B, T, C = 4, 1024, 1024
NH = 16
HD = C // NH  # 64
NHL = 8  # heads per core
LAMBDA_INIT = 0.8 - 0.6 * math.exp(-0.3 * 1.0)
EPS = 1e-5
SCALE = 1.0 / math.sqrt(HD)
N_CORES = 8
NKT = T // 128  # 8 token tiles
NKC = C // 128  # 8 contraction tiles
HEAD_ORDER = [1, 3, 5, 7, 0, 2, 4, 6]  # odd slots first: no DMA on the tail


def _const(nc, val, shape):
    return nc.const_aps.tensor(val, shape)


def _dram_tensors(nc):
    x_d = nc.dram_tensor("xbT", [C, T], BF16, kind="ExternalInput").ap()
    wq_d = nc.dram_tensor("wq", [C, 1024], BF16, kind="ExternalInput").ap()
    wk_d = nc.dram_tensor("wk", [C, 1024], BF16, kind="ExternalInput").ap()
    wv_d = nc.dram_tensor("wv", [C, 512], BF16, kind="ExternalInput").ap()
    wc_d = nc.dram_tensor("wc", [512, C], F32, kind="ExternalInput").ap()
    neglam_d = nc.dram_tensor("neglam", [1, 64], F32, kind="ExternalInput").ap()
    out_d = nc.dram_tensor("outp", [T, C], F32, kind="ExternalOutput").ap()
    return (x_d, wq_d, wk_d, wv_d, wc_d, neglam_d, out_d)


def build_program(n_iters: int = 1):
    nc = bacc.Bacc("TRN2", target_bir_lowering=False, debug=False)
    dram = _dram_tensors(nc)

    with tile.TileContext(nc) as tc, ExitStack() as bctx:
        pers = _alloc_persistent(nc, tc, bctx, dram)
        if n_iters == 1:
            _emit_iteration(nc, tc, pers, dram)
        else:
            with tc.For_i(0, n_iters, 1):
                _emit_iteration(nc, tc, pers, dram)

    nc.compile()
    return nc


def _alloc_persistent(nc, tc, bctx, dram):
    """Pools + one-time init (consts, causal-zero p regions, vaug ones col)."""
    neglam_d = dram[5]
    lp = bctx.enter_context(tc.tile_pool(name="long", bufs=1))
    qk = bctx.enter_context(tc.tile_pool(name="qk", bufs=1))
    oe = bctx.enter_context(tc.tile_pool(name="oe", bufs=1))
    yout = bctx.enter_context(tc.tile_pool(name="yn", bufs=1))
    pp = bctx.enter_context(tc.tile_pool(name="pp", bufs=1))

    p = {"lp": lp, "qk": qk, "oe": oe, "yout": yout, "pp": pp}

    p["neglam_f"] = lp.tile([1, 64], F32, tag="neglam_f", name="neglam_f")
    # lambda is iteration-invariant: load once
    p["onesr"] = lp.tile([1, 64], F32R, tag="onesr", name="onesr")
    nc.vector.tensor_copy(p["onesr"][:], _const(nc, 1.0, (1, 64)))
    p["actscr"] = lp.tile([1, 1], F32, tag="actscr", name="actscr")
    p["ones128"] = lp.tile([64, 128], F32R, tag="ones128", name="ones128")
    nc.vector.tensor_copy(p["ones128"][:], _const(nc, 1.0, (64, 128)))
    nc.sync.dma_start(p["neglam_f"][:], neglam_d)
    p["neglamr"] = lp.tile([1, 64], F32R, tag="neglamr", name="neglamr")
    nc.vector.tensor_copy(p["neglamr"][:], p["neglam_f"][:])

    p["wc_t"] = oe.tile([128, 4, C], F32, tag="wc_t", name="wc_t")
    p["wcs_t"] = oe.tile([128, 4, C], F32R, tag="wcs_t", name="wcs_t")
    p["vaug"] = [
        lp.tile([128, NHL, HD + 1], BF16, tag=f"vaug{t}", name=f"vaug{t}") for t in range(NKT)
    ]
    for tt in range(NKT):
        nc.vector.tensor_copy(p["vaug"][tt][:, :, HD : HD + 1], _const(nc, 1.0, (128, NHL, 1)))
    p["QT"] = [qk.tile([128, T], BF16, tag=f"qt{m}", name=f"qt{m}") for m in range(NKC)]
    p["KT"] = [qk.tile([128, T], BF16, tag=f"kt{m}", name=f"kt{m}") for m in range(NKC)]
    p["yTn"] = [yout.tile([128, T], F32R, tag=f"ytn{k}", name=f"ytn{k}") for k in range(4)]

    # p tiles: explicit double buffer (parity = head % 2), bf16.
    # Region [0, d0) is the causal-mask zero block: written only here, read by
    # every u matmul, so the zeros persist across loop iterations.
    p["pbuf"] = []
    for par in range(2):
        p1 = [
            pp.tile([128, T - (i // 4) * 512], BF16, tag=f"p1_{par}_{i}", name=f"p1_{par}_{i}")
            for i in range(NKT)
        ]
        p2 = [
            pp.tile([128, T - (i // 4) * 512], BF16, tag=f"p2_{par}_{i}", name=f"p2_{par}_{i}")
            for i in range(NKT)
        ]
        p["pbuf"].append((p1, p2))
        for i in range(NKT):
            d0 = i * 128 - (i // 4) * 512
            if d0 > 0:
                nc.vector.memset(p1[i][:, 0:d0], 0.0)
                nc.vector.memset(p2[i][:, 0:d0], 0.0)

    # x / projection weights: persistent bf16 tiles so the next iteration's
    # input DMAs only wait on this iteration's projection reads (prefetch)
    xw = bctx.enter_context(tc.tile_pool(name="xw", bufs=1))
    p["x_t"] = xw.tile([128, NKC, T], BF16, tag="x_t", name="x_t")
    p["wqk_t"] = xw.tile([128, NKC, 1024], BF16, tag="wqk_t", name="wqk_t")
    p["wv_t"] = xw.tile([128, NKC, 512], BF16, tag="wv_t", name="wv_t")
    return p


def _emit_iteration(nc, tc, pers, dram):
    x_d, wq_d, wk_d, wv_d, wc_d, neglam_d, out_d = dram
    QT, KT, vaug, yTn = pers["QT"], pers["KT"], pers["vaug"], pers["yTn"]
    wc_t, wcs_t, neglamr = pers["wc_t"], pers["wcs_t"], pers["neglamr"]
    onesr, ones128, pbuf = pers["onesr"], pers["ones128"], pers["pbuf"]

    # preload the exp table off the critical path (ACT idles during phase A/B)
    nc.scalar.activation(pers["actscr"][:], onesr[0:1, 0:1], AF.Exp)

    # ---------------- phase A+B: projections (Q -> V -> K) ----------------
    with ExitStack() as ab:
        psb = ab.enter_context(tc.tile_pool(name="psb", bufs=3, space="PSUM"))

        x_t, wqk_t, wv_t = pers["x_t"], pers["wqk_t"], pers["wv_t"]
        x_r = x_d.rearrange("(k p) n -> p k n", p=128)
        nc.sync.dma_start(x_t[:, 0:4, :], x_r[:, 0:4, :])
        nc.sync.dma_start(x_t[:, 4:8, :], x_r[:, 4:8, :])

        wq_r = wq_d.rearrange("(k p) n -> p k n", p=128)
        nc.sync.dma_start(wqk_t[:, 0:4, :], wq_r[:, 0:4, :])
        nc.sync.dma_start(wqk_t[:, 4:8, :], wq_r[:, 4:8, :])
        nc.sync.dma_start(wv_t[:], wv_d.rearrange("(k p) n -> p k n", p=128))
        # wc after wv: its WAR (previous out-proj) resolves at the iteration
        # boundary, so it must not head-of-line block the x/wq prefetch
        nc.sync.dma_start(wc_t[:], wc_d.rearrange("(k p) n -> p k n", p=128))

        def proj_qk(dest):
            for m in range(NKC):
                pq = psb.tile([128, T], F32, tag="proj", bufs=3)
                for c0 in range(0, T, 512):
                    for k in range(NKC):
                        nc.tensor.matmul(
                            pq[:, c0 : c0 + 512],
                            wqk_t[:, k, m * 128 : (m + 1) * 128],
                            x_t[:, k, c0 : c0 + 512],
                            start=(k == 0),
                            stop=(k == NKC - 1),
                        )
                if m % 2 == 0:
                    nc.scalar.copy(dest[m][:], pq[:])
                else:
                    nc.vector.tensor_copy(dest[m][:], pq[:])

        proj_qk(QT)

        # V projection into Vaug while wk streams in
        wk_r = wk_d.rearrange("(k p) n -> p k n", p=128)
        for tt in range(NKT):
            pv = psb.tile([128, 512], F32, tag="projv", bufs=2)
            for k in range(NKC):
                nc.tensor.matmul(
                    pv[:],
                    x_t[:, k, tt * 128 : (tt + 1) * 128],
                    wv_t[:, k, :],
                    start=(k == 0),
                    stop=(k == NKC - 1),
                )
            if tt % 2 == 0:
                nc.vector.tensor_copy(
                    vaug[tt][:, :, 0:HD], pv[:].rearrange("p (h d) -> p h d", h=NHL)
                )
            else:
                nc.scalar.copy(
                    vaug[tt][:, :, 0:HD], pv[:].rearrange("p (h d) -> p h d", h=NHL)
                )
        nc.sync.dma_start(wqk_t[:, 0:4, :], wk_r[:, 0:4, :])
        nc.sync.dma_start(wqk_t[:, 4:8, :], wk_r[:, 4:8, :])

        proj_qk(KT)

    # ---------------- phase C: attention per head ----------------
    with ExitStack() as cc_:
        yt = cc_.enter_context(tc.tile_pool(name="yt", bufs=1))
        sm = cc_.enter_context(tc.tile_pool(name="sm", bufs=2))
        pss = cc_.enter_context(tc.tile_pool(name="pss", bufs=1, space="PSUM"))
        psu = cc_.enter_context(tc.tile_pool(name="psu", bufs=1, space="PSUM"))
        psr = cc_.enter_context(tc.tile_pool(name="psr", bufs=1, space="PSUM"))

        # varW[p, kk] = var+eps of head 2*kk + p//64, replicated in its slot
        varW = sm.tile([128, 4], F32, tag="varW", bufs=1)
        yT_heads = {}

        def emit_score_mms(j, i):
            c0 = (i // 4) * 512
            w = T - c0
            s1 = pss.tile([128, w], F32, tag="s1", name=f"s1_{j}_{i}")
            s2 = pss.tile([128, w], F32, tag="s2", name=f"s2_{j}_{i}")
            for cb in range(0, w, 512):
                nc.tensor.matmul(
                    s1[:, cb : cb + 512],
                    KT[j][0:64, i * 128 : (i + 1) * 128],
                    QT[j][0:64, c0 + cb : c0 + cb + 512],
                    start=True,
                    stop=True,
                    tile_position=(0, 0),
                )
                nc.tensor.matmul(
                    s2[:, cb : cb + 512],
                    KT[j][64:128, i * 128 : (i + 1) * 128],
                    QT[j][64:128, c0 + cb : c0 + cb + 512],
                    start=True,
                    stop=True,
                    tile_position=(64, 0),
                )
            return s1, s2

        def emit_scores(j, par, s_pre=None):
            p1_t, p2_t = pbuf[par]
            for i in range(NKT):
                c0 = (i // 4) * 512
                d0 = i * 128 - c0
                if i == 0 and s_pre is not None:
                    s1, s2 = s_pre
                else:
                    s1, s2 = emit_score_mms(j, i)
                nc.scalar.activation(p1_t[i][:, d0:], s1[:, d0:], AF.Exp, scale=SCALE)
                nc.scalar.activation(p2_t[i][:, d0:], s2[:, d0:], AF.Exp, scale=SCALE)
                for pt in (p1_t, p2_t):
                    nc.gpsimd.affine_select(
                        out=pt[i][:, d0 : d0 + 128],
                        in_=pt[i][:, d0 : d0 + 128],
                        compare_op=OP.is_ge,
                        fill=0.0,
                        base=0,
                        pattern=[[1, 128]],
                        channel_multiplier=-1,
                    )
            yT_heads[j] = yt.tile([64, T], F32, tag="yT", bufs=3, name=f"yTh{j}")

        def emit_u(j, par, c):
            p1, p2 = pbuf[par]
            yT_h = yT_heads[j]
            ilast = min(NKT, (c + 1) * 4) - 1
            # u1 | u2 side by side in one 2-bank PSUM tile so the combine is
            # one reciprocal, one staging copy, and one multiply
            u12 = psu.tile([HD + 1, 1024], F32, tag="u1", name=f"u12_{j}_{c}")
            for i in range(ilast + 1):
                lo = c * 512 - (i // 4) * 512
                nc.tensor.matmul(
                    u12[:, 0:512], vaug[i][:, j, :], p1[i][:, lo : lo + 512],
                    start=(i == 0), stop=(i == ilast),
                )
            for i in range(ilast + 1):
                lo = c * 512 - (i // 4) * 512
                nc.tensor.matmul(
                    u12[:, 512:1024], vaug[i][:, j, :], p2[i][:, lo : lo + 512],
                    start=(i == 0), stop=(i == ilast),
                )
            rr12 = sm.tile([1, 1024], F32R, tag="rr1", bufs=1, name=f"rr12_{j}_{c}")
            with nc.allow_low_precision(reason="f32r denominators feed a broadcast matmul"):
                nc.vector.reciprocal(rr12[:], u12[64:65, :])
            # broadcast 1/den across 64 partitions via contraction-1 matmuls;
            # -lam folds into the u2 half via the neglam stationary
            R12s = psr.tile([64, 1024], F32, tag="R1", name=f"R12_{j}_{c}")
            nc.tensor.matmul(R12s[:, 0:512], onesr[:], rr12[:, 0:512], start=True, stop=True)
            nc.tensor.matmul(R12s[:, 512:1024], neglamr[:], rr12[:, 512:1024], start=True, stop=True)
            # TensorTensor may read at most one PSUM input: stage R in SBUF
            R12b = sm.tile([64, 1024], F32, tag="R1b", bufs=1, name=f"R12b_{j}_{c}")
            nc.scalar.copy(R12b[:], R12s[:])
            t12 = sm.tile([64, 1024], F32, tag="t1", bufs=1, name=f"t12_{j}_{c}")
            nc.vector.tensor_tensor(t12[:], u12[0:HD, :], R12b[:], OP.mult)
            # gpsimd cannot read PSUM; it gets the SBUF-only add
            nc.gpsimd.tensor_tensor(
                yT_h[:, c * 512 : (c + 1) * 512], t12[:, 0:512], t12[:, 512:1024], OP.add
            )

        def emit_stats_norm(j):
            yT_h = yT_heads[j]
            bstats = sm.tile([64, 2, 6], F32, tag="bst", name=f"bst_{j}")
            for si in range(2):
                nc.vector.bn_stats(out=bstats[:, si, :], in_=yT_h[:, si * 512 : (si + 1) * 512])
            mv = sm.tile([64, 2], F32, tag="mv", name=f"mv_{j}")
            nc.vector.bn_aggr(out=mv[:], in_=bstats[:])
            # st = (mean, var + mean^2) per partition; sum across partitions
            st = sm.tile([64, 2], F32R, tag="st", name=f"st_{j}")
            m2p = sm.tile([64, 1], F32, tag="m2p", name=f"m2p_{j}")
            nc.vector.tensor_tensor(m2p[:], mv[:, 0:1], mv[:, 0:1], OP.mult)
            nc.vector.tensor_tensor(st[:, 1:2], mv[:, 1:2], m2p[:], OP.add)
            nc.vector.tensor_copy(st[:, 0:1], mv[:, 0:1])
            pstat = psu.tile([128, 2], F32, tag="u1", name=f"pstat_{j}")
            nc.tensor.matmul(pstat[:], ones128[:], st[:], start=True, stop=True)
            nm128 = sm.tile([128, 1], F32, tag="nm128", name=f"nm_{j}")
            nc.vector.tensor_scalar_mul(nm128[:], pstat[:, 0:1], -1.0 / 64.0)
            m2 = sm.tile([128, 1], F32, tag="m2", name=f"m2_{j}")
            nc.vector.tensor_tensor(m2[:], nm128[:], nm128[:], OP.mult)
            ve = sm.tile([128, 1], F32, tag="ve", name=f"ve_{j}")
            nc.vector.tensor_scalar(
                out=ve[:], in0=pstat[:, 1:2], scalar1=1.0 / 64.0, scalar2=EPS,
                op0=OP.mult, op1=OP.add,
            )
            nc.vector.tensor_tensor(ve[:], ve[:], m2[:], OP.subtract)
            lo = (j % 2) * 64
            nc.vector.tensor_copy(varW[lo : lo + 64, j // 2 : j // 2 + 1], ve[lo : lo + 64, :])
            z = sm.tile([64, 1], F32, tag="z", name=f"z_{j}")
            nc.vector.reciprocal(z[:], ve[0:64, :])
            # normalize to (y - mean)/(var+eps); the sqrt(var+eps)*(1-lam_init)
            # factor lands in the Wc row scale at the end
            if j % 2 == 0:
                nc.vector.tensor_scalar(
                    out=yTn[j // 2][0:64, :], in0=yT_h[:],
                    scalar1=nm128[0:64, :], scalar2=z[:], op0=OP.add, op1=OP.mult,
                )
            else:
                ymv = sm.tile([64, T], F32R, tag="ymv", bufs=1, name=f"ymv_{j}")
                nc.vector.tensor_scalar(
                    out=ymv[:], in0=yT_h[:],
                    scalar1=nm128[0:64, :], scalar2=z[:], op0=OP.add, op1=OP.mult,
                )
                nc.scalar.dma_start(yTn[j // 2][64:128, :], ymv[:])

        # software-pipelined head loop
        for idx, j in enumerate(HEAD_ORDER):
            par = idx % 2
            s_pre = None
            if idx > 0:
                s_pre = emit_score_mms(j, 0)
                emit_u(HEAD_ORDER[idx - 1], 1 - par, 1)
            emit_scores(j, par, s_pre)
            emit_u(j, par, 0)
            if idx > 0:
                emit_stats_norm(HEAD_ORDER[idx - 1])
        emit_u(HEAD_ORDER[-1], 1, 1)
        # fold sqrt(var+eps)*(1-lam_init) into Wc rows. Heads for kk=0..2 are
        # done; sqrt them (one table load) + scale on DVE while the last
        # head's u/stats run, leaving only kk=3 on the tail.
        c2 = (1.0 - LAMBDA_INIT) ** 2
        srstd = sm.tile([128, 4], F32, tag="srstd", bufs=1)
        nc.scalar.activation(srstd[:, 0:3], varW[:, 0:3], AF.Sqrt, scale=c2)
        for kk in range(3):
            nc.vector.tensor_scalar_mul(wcs_t[:, kk, :], wc_t[:, kk, :], srstd[:, kk : kk + 1])
        emit_stats_norm(HEAD_ORDER[-1])
        nc.scalar.activation(srstd[:, 3:4], varW[:, 3:4], AF.Sqrt, scale=c2)
        nc.vector.tensor_scalar_mul(wcs_t[:, 3, :], wc_t[:, 3, :], srstd[:, 3:4])

    # ---------------- phase E: output projection ----------------
    with ExitStack() as ee:
        ob = ee.enter_context(tc.tile_pool(name="ob", bufs=4))
        pso = ee.enter_context(tc.tile_pool(name="pso", bufs=3, space="PSUM"))
        for m in range(NKC):
            po = pso.tile([128, C], F32, tag="o")
            for c0 in range(0, C, 512):
                for kk in range(4):
                    nc.tensor.matmul(
                        po[:, c0 : c0 + 512],
                        yTn[kk][:, m * 128 : (m + 1) * 128],
                        wcs_t[:, kk, c0 : c0 + 512],
                        start=(kk == 0),
                        stop=(kk == 3),
                    )
            osb = ob.tile([128, C], F32, tag="osb")
            if m % 2 == 0:
                nc.vector.tensor_copy(osb[:], po[:])
            else:
                nc.scalar.copy(osb[:], po[:])
            # output stores ride the gpsimd queue (idle at phase E) so they
            # neither block the SP prefetch nor delay the last osb copies
            nc.gpsimd.dma_start(out_d[m * 128 : (m + 1) * 128, :], osb[:])


_PROGRAM_CACHE = {}


def get_program(n_iters: int = 1):
    if n_iters not in _PROGRAM_CACHE:
        _PROGRAM_CACHE[n_iters] = build_program(n_iters)
    return _PROGRAM_CACHE[n_iters]


def make_in_maps(x, Wq, Wk, Wv, Wc, lambda_q1, lambda_k1, lambda_q2, lambda_k2):
    lam = (
        math.exp(float(np.sum(lambda_q1.astype(np.float64) * lambda_k1.astype(np.float64))))
        - math.exp(float(np.sum(lambda_q2.astype(np.float64) * lambda_k2.astype(np.float64))))
        + LAMBDA_INIT
    )
    neglam = np.full((1, 64), -lam, dtype=np.float32)
    in_maps = []
    for core in range(N_CORES):
        b, g = core // 2, core % 2
        in_maps.append(
            {
                "xbT": np.ascontiguousarray(x[b].T).astype(BF16_NP),
                "wq": np.ascontiguousarray(Wq[:, g * 1024 : (g + 1) * 1024]).astype(BF16_NP),
                "wk": np.ascontiguousarray(Wk[:, g * 1024 : (g + 1) * 1024]).astype(BF16_NP),
                "wv": np.ascontiguousarray(Wv[:, g * 512 : (g + 1) * 512]).astype(BF16_NP),
                "wc": np.ascontiguousarray(Wc[g * 512 : (g + 1) * 512, :]),
                "neglam": neglam,
            }
        )
    return in_maps


def kernel(x, Wq, Wk, Wv, Wc, lambda_q1, lambda_k1, lambda_q2, lambda_k2):
    x = np.asarray(x, dtype=np.float32)
    in_maps = make_in_maps(
        x,
        np.asarray(Wq, np.float32),
        np.asarray(Wk, np.float32),
        np.asarray(Wv, np.float32),
        np.asarray(Wc, np.float32),
        np.asarray(lambda_q1, np.float32),
        np.asarray(lambda_k1, np.float32),
        np.asarray(lambda_q2, np.float32),
        np.asarray(lambda_k2, np.float32),
    )
    nc = get_program(1)
    res = run_bass_kernel_spmd(nc, in_maps, list(range(N_CORES)))
    out = np.empty((B, T, C), dtype=np.float32)
    for b in range(B):
        out[b] = res.results[2 * b]["outp"] + res.results[2 * b + 1]["outp"]
    return out
